# revision 13
# baseline (speedup 1.0000x reference)
"""Trainium2 Bass kernel for nn_CriticNetwork (GCN message passing + critic MLP).

Strategy (8 NeuronCores, SPMD, no collectives):
  - Only agg[agent_idx] rows are consumed downstream, so message passing is
    pruned to edges whose destination is an agent node, and the GCN transform
    is moved after aggregation: A_hat @ (x W) == (A_hat @ x) W.
  - Agents are globally sorted by indegree and dealt round-robin to the 8
    cores, so every core sees an identical degree profile. The host
    materializes each core's (dinv-prescaled, bf16) edge-source rows
    feature-major into a dense slot tensor E with a per-256-agent-block slot
    count K (tight padding), streamed in with large sequential DMAs.
  - Aggregation runs as bf16 pairwise add trees on the vector engine (2x DVE
    mode) with a minority share on gpsimd (no PSUM port, 0.42 add efficiency).
  - Head (critic MLP) runs feature-major with bf16 matmul operands (f32 PSUM
    accumulation). LayerNorm mean-centering is folded into W1/W2 host-side;
    LN1's sum-of-squares comes from the quadratic form z^T(W1f W1f^T)z +
    2(W1f b1c)^T z + const. LN1's rstd is absorbed into LN2 EXACTLY:
    with v := lp + b2c*std1 (u = v/std1), t = u*rstd2_ref = v/sqrt(
    mean_f v^2 + eps*var1til), so rstd1 is never materialized.
  - Emission is software-pipelined: group 0 is processed as two half-width
    passes so the PE starts ~4us in (p-state ramp), each pass's tail
    (wa/wq/t2/sa) is deferred into the next pass's matmul stream so the
    tensor engine never idles at group boundaries.
"""
import os
import sys

sys.path.insert(0, "/opt/trn_rl_repo")

import numpy as np
import ml_dtypes

import concourse.bass as bass
import concourse.tile as tile
import concourse.mybir as mybir
from concourse import bacc
from concourse.bass_utils import run_bass_kernel_spmd

# ---- problem constants (hardcoded per spec) ----
N_NODES = 50000
DIM = 128          # IN_DIM
HID = 256
F1 = 1024
F2 = 512
NACT = 64
N_EDGES = 800000
N_AGENTS = 16384
N_CORES = 8
PA = N_AGENTS // N_CORES      # 2048 agents per core
BLK = 256                     # slot-count granularity (agents per K-block)
NB = PA // BLK                # 8 K-blocks per core
EPS = 1e-5
# head passes: (col_start, width, [K-block ids]) -- first group split in two
# half-width passes so the tensor engine starts early during DMA warmup.
PASSES = [(0, 256, (0,)), (256, 256, (1,)), (512, 512, (2, 3)),
          (1024, 512, (4, 5)), (1536, 512, (6, 7))]
S1_ON_D = (5, 6)              # L1-relu feature tiles drained on vector engine

F32 = mybir.dt.float32
F32R = mybir.dt.float32r
BF16 = mybir.dt.bfloat16
AF = mybir.ActivationFunctionType
OP = mybir.AluOpType

_KERNEL_CACHE = {}


def _preprocess(x, action, W_gcn, b_gcn, W1, b1, g1, beta1, W2, b2, g2, beta2,
                Wa, ba, Wq, bq, edge_index, agent_idx):
    f32 = np.float32
    x = np.asarray(x, f32); action = np.asarray(action, f32)
    edge_index = np.asarray(edge_index); agent_idx = np.asarray(agent_idx)
    W_gcn = np.asarray(W_gcn, f32); b_gcn = np.asarray(b_gcn, f32)
    W1 = np.asarray(W1, f32); b1 = np.asarray(b1, f32)
    g1 = np.asarray(g1, f32); beta1 = np.asarray(beta1, f32)
    W2 = np.asarray(W2, f32); b2 = np.asarray(b2, f32)
    g2 = np.asarray(g2, f32); beta2 = np.asarray(beta2, f32)
    Wa = np.asarray(Wa, f32); ba = np.asarray(ba, f32)
    Wq = np.asarray(Wq, f32); bq = np.asarray(bq, f32)

    assert np.all(beta1 == 0.0) and np.all(g1 > 0.0), \
        "kernel fast path requires beta1==0 and g1>0 (module init guarantees this)"

    N = N_NODES
    loops = np.arange(N, dtype=edge_index.dtype)
    src_all = np.concatenate([edge_index[0], loops])
    dst_all = np.concatenate([edge_index[1], loops])
    deg = np.bincount(dst_all, minlength=N).astype(np.int64)
    dinv = (1.0 / np.sqrt(np.maximum(deg, 1.0))).astype(f32)

    order = np.argsort(dst_all, kind="stable")
    src_sorted = src_all[order]
    starts = np.searchsorted(dst_all[order], np.arange(N + 1))

    # global indegree sort + round-robin deal: rank r -> core r%8, pos r//8.
    ind_all = deg[agent_idx]
    rank = np.argsort(ind_all, kind="stable")
    # shared per-block K (identical across cores by construction)
    Kb = []
    for b in range(NB):
        mx = int(ind_all[rank[8 * BLK * b: 8 * BLK * (b + 1)]].max())
        Kb.append(max(2, ((mx + 1) // 2) * 2))
    boff = np.concatenate([[0], np.cumsum(Kb)]).astype(int)
    tot_cols = int(boff[-1]) * BLK

    # prescaled node features, plus a zero pad row for empty slots
    xsf = np.zeros((N + 1, DIM), f32)
    xsf[:N] = x * dinv[:, None]

    E_list, actT_list, rows_list = [], [], []
    for c in range(N_CORES):
        rows = rank[np.arange(PA) * 8 + c]          # original agent rows
        ag = agent_idx[rows]
        dd = dinv[ag]
        Ec = np.empty((128, tot_cols), ml_dtypes.bfloat16)
        for b in range(NB):
            K = Kb[b]
            tbl = np.full((K, BLK), N, np.int64)
            for j in range(BLK):
                a = int(ag[b * BLK + j]); d = int(deg[a]); s = starts[a]
                tbl[:d, j] = src_sorted[s:s + d]
            blk = (xsf[tbl] * dd[b * BLK:(b + 1) * BLK][None, :, None])
            Ec[:, boff[b] * BLK:(boff[b] + K) * BLK] = (
                blk.transpose(2, 0, 1).reshape(128, K * BLK)
            ).astype(ml_dtypes.bfloat16)
        E_list.append(Ec)
        actp = action[rows].T                        # [64, PA]
        actT_list.append(np.ascontiguousarray(np.concatenate(
            [actp, np.ones((1, PA), f32)], axis=0)).astype(ml_dtypes.bfloat16))
        rows_list.append(rows)

    # ---- weight folding (exact algebra) ----
    w1m = W1.mean(axis=1)
    W1f = W1 - w1m[:, None]
    b1c = b1 - b1.mean()
    W2g = g1[:, None] * W2
    w2gm = W2g.mean(axis=1)
    W2f = W2g - w2gm[:, None]
    b2c = b2 - b2.mean()
    bb = ba + beta2
    M1 = (W1f @ W1f.T).astype(f32)
    cvec = (2.0 * (W1f @ b1c)).astype(f32)
    c1const = float(np.sum(b1c * b1c))

    def ktile_pack(W, kt, fdim):   # [kt*128, fdim] -> [128, kt*fdim]
        return np.ascontiguousarray(
            W.reshape(kt, 128, fdim).transpose(1, 0, 2).reshape(128, kt * fdim))

    # w2 packed c2-major: [128, c2*1024 + k8*128] so the DMA can stream the
    # c2=0 stationaries first (L2 of pass 0 starts before the full load).
    w2p = ktile_pack(W2f, 8, F2).reshape(128, 8, 4, 128).transpose(
        0, 2, 1, 3).reshape(128, 8 * F2)

    bf = ml_dtypes.bfloat16
    weights = {
        "wgcn": W_gcn.astype(bf),                               # [128, 256]
        "w1": ktile_pack(W1f, 2, F1).astype(bf),                # [128, 2048]
        "w2": np.ascontiguousarray(w2p).astype(bf),             # [128, 4096]
        "m1": ktile_pack(M1, 2, HID).astype(bf),                # [128, 512]
        "wa": np.ascontiguousarray(
            np.concatenate([Wa, bb[None, :]], axis=0)).astype(bf),  # [65, 512]
        "wq": np.ascontiguousarray(Wq.reshape(4, 128).T).astype(bf),  # [128, 4]
        "cols": np.ascontiguousarray(np.concatenate([
            b_gcn.reshape(2, 128).T,      # [:, 0:2]   bgcn
            b1c.reshape(8, 128).T,        # [:, 2:10]  b1c
            cvec.reshape(2, 128).T,       # [:, 10:12] cvec
            b2c.reshape(4, 128).T,        # [:, 12:16] b2c
            g2.reshape(4, 128).T,         # [:, 16:20] g2
        ], axis=1).astype(f32)),
        "onesmat_in": np.ones((128, 128), bf),
    }
    meta = dict(Kb=tuple(int(k) for k in Kb),
                boff=tuple(int(o) for o in boff),
                tot_cols=tot_cols, bq=float(bq[0]), c1const=c1const)
    percore = dict(E=E_list, actT=actT_list)
    return weights, percore, rows_list, meta


def _build(meta):
    Kb = meta["Kb"]; boff = meta["boff"]
    tot_cols = meta["tot_cols"]; bq = meta["bq"]; c1const = meta["c1const"]

    nc = bacc.Bacc("TRN2", target_bir_lowering=False, debug=False,
                   num_devices=N_CORES, num_swdge_queues=4)
    dram = {}
    def din(name, shape, dt):
        dram[name] = nc.dram_tensor(name, shape, dt, kind="ExternalInput").ap()
        return dram[name]

    E_d = din("E", [128, tot_cols], BF16)
    actT_d = din("actT", [NACT + 1, PA], BF16)
    wgcn_d = din("wgcn", [128, HID], BF16)
    w1_d = din("w1", [128, 2 * F1], BF16)
    w2_d = din("w2", [128, 8 * F2], BF16)
    m1_d = din("m1", [128, 2 * HID], BF16)
    wa_d = din("wa", [NACT + 1, F2], BF16)
    wq_d = din("wq", [128, 4], BF16)
    cols_d = din("cols", [128, 20], F32)
    onesm_d = din("onesmat_in", [128, 128], BF16)
    OUT = nc.dram_tensor("q", [1, PA], F32, kind="ExternalOutput").ap()

    with tile.TileContext(nc) as tc:
        with tc.tile_pool(name="w", bufs=1) as wp, \
             tc.tile_pool(name="eb", bufs=1) as ep, \
             tc.tile_pool(name="prp", bufs=3) as prp, \
             tc.tile_pool(name="csp", bufs=12) as csp, \
             tc.tile_pool(name="zp", bufs=3) as zp, \
             tc.tile_pool(name="s1p", bufs=9) as s1p, \
             tc.tile_pool(name="dp", bufs=3) as dp, \
             tc.tile_pool(name="uup", bufs=5) as uup, \
             tc.tile_pool(name="u2p", bufs=3) as u2p, \
             tc.tile_pool(name="vec", bufs=6) as vec, \
             tc.tile_pool(name="bcp", bufs=3) as bcp, \
             tc.tile_pool(name="tlp", bufs=4) as tlp, \
             tc.tile_pool(name="wvp", bufs=4) as wvp, \
             tc.tile_pool(name="sap", bufs=8) as sap, \
             tc.tile_pool(name="qvp", bufs=2) as qvp, \
             tc.tile_pool(name="ps", bufs=1, space="PSUM") as pp:

            T = nc.tensor; S = nc.scalar; D = nc.vector; G = nc.gpsimd
            SY = nc.sync

            # ---------- DMA: E blocks resident, spread across queues ----------
            # early phase is DMA-bandwidth-bound: each E block is half-split
            # across the sync + scalar queues in strict consumption order, so
            # block b lands as early as the aggregate bandwidth allows.
            # weights ride the gpsimd queue in first-use order.
            eb = [ep.tile([128, Kb[b] * BLK], BF16, name=f"eb{b}")
                  for b in range(NB)]
            wgcn = wp.tile([128, HID], BF16); SY.dma_start(wgcn[:], wgcn_d[:])
            cols = wp.tile([128, 20], F32); S.dma_start(cols[:], cols_d[:])
            def eslice(b, lo, hi):
                return (eb[b][:, lo * BLK:hi * BLK],
                        E_d[:, (boff[b] + lo) * BLK:(boff[b] + hi) * BLK])
            for b in range(NB):
                K = Kb[b]
                h1 = min(max(4, (K // 2 + 3) // 4 * 4), K)
                SY.dma_start(*eslice(b, 0, h1))
                if h1 < K:
                    S.dma_start(*eslice(b, h1, K))
            actT = wp.tile([NACT + 1, PA], BF16); G.dma_start(actT[:], actT_d[:])
            wa = wp.tile([NACT + 1, F2], BF16); G.dma_start(wa[:], wa_d[:])
            wq = wp.tile([128, 4], BF16); G.dma_start(wq[:], wq_d[:])
            w1 = wp.tile([128, 2 * F1], BF16); G.dma_start(w1[:], w1_d[:])
            m1 = wp.tile([128, 2 * HID], BF16); G.dma_start(m1[:], m1_d[:])
            onesm = wp.tile([128, 128], BF16); G.dma_start(onesm[:], onesm_d[:])
            w2 = wp.tile([128, 8 * F2], BF16)
            for c2 in range(4):
                G.dma_start(w2[:, c2 * 1024:(c2 + 1) * 1024],
                            w2_d[:, c2 * 1024:(c2 + 1) * 1024])

            bgcn = cols[:, 0:2]
            b1c = cols[:, 2:10]
            cvec = cols[:, 10:12]
            b2c = cols[:, 12:16]
            g2c = cols[:, 16:20]
            agg = wp.tile([128, PA], BF16)       # agg^T, feature-major

            # ---------- aggregation: bf16 pairwise trees, D majority ----------
            def agg_ops(b, pattern):
                """Closures (dependency-ordered) summing E block b -> agg."""
                K = Kb[b]; et = eb[b]
                av = agg[:, b * BLK:(b + 1) * BLK]
                ops = []
                sums = {"D": [], "G": []}
                nch = K // 4
                for q in range(nch):
                    eng = pattern[q % len(pattern)]
                    nm = "D" if eng is D else "G"
                    c3 = et[:, 4 * q * BLK:(4 * q + 4) * BLK].rearrange(
                        "p (s e) -> p s e", e=BLK)
                    pr = prp.tile([128, 2 * BLK], BF16, tag="pr" + nm)
                    p3 = pr[:].rearrange("p (s e) -> p s e", e=BLK)
                    ops.append(lambda eng=eng, p3=p3, c3=c3: eng.tensor_tensor(
                        p3, c3[:, 0::2, :], c3[:, 1::2, :], OP.add))
                    cs = csp.tile([128, BLK], BF16, tag="cs" + nm)
                    ops.append(lambda eng=eng, cs=cs, pr=pr: eng.tensor_tensor(
                        cs[:], pr[:, :BLK], pr[:, BLK:], OP.add))
                    sums[nm].append(cs)
                if K - 4 * nch == 2:
                    eng = pattern[nch % len(pattern)]
                    nm = "D" if eng is D else "G"
                    c3 = et[:, 4 * nch * BLK:(4 * nch + 2) * BLK].rearrange(
                        "p (s e) -> p s e", e=BLK)
                    cs = csp.tile([128, BLK], BF16, tag="cs" + nm)
                    ops.append(lambda eng=eng, cs=cs, c3=c3: eng.tensor_tensor(
                        cs[:], c3[:, 0, :], c3[:, 1, :], OP.add))
                    sums[nm].append(cs)
                parts = []
                for nm, eng in (("D", D), ("G", G)):
                    lst = sums[nm]
                    while len(lst) > 1:
                        nxt = []
                        for i in range(0, len(lst) - 1, 2):
                            t = csp.tile([128, BLK], BF16, tag="cs" + nm)
                            ops.append(lambda eng=eng, t=t, a=lst[i],
                                       b2=lst[i + 1]: eng.tensor_tensor(
                                           t[:], a[:], b2[:], OP.add))
                            nxt.append(t)
                        if len(lst) % 2:
                            nxt.append(lst[-1])
                        lst = nxt
                    if lst:
                        parts.append(lst[0])
                if len(parts) == 2:
                    ops.append(lambda a=parts[0], b2=parts[1]: D.tensor_tensor(
                        av, a[:], b2[:], OP.add))
                else:
                    ops.append(lambda a=parts[0]: D.tensor_scalar(
                        av, a[:], 0.0, None, OP.add))
                return ops

            extras_by_pass = {
                0: agg_ops(2, (D, D, D, G)),
                1: agg_ops(3, (D, D, D, G)),
                2: agg_ops(4, (D, D, D, G)) + agg_ops(5, (D, D, D, G)),
                3: agg_ops(6, (D, D, D, G)) + agg_ops(7, (D, D, D, G)),
                4: [],
            }

            # ---------- head pass emitter (software pipelined) ----------
            def emit_pass(pi, deferred):
                s0, W, _ = PASSES[pi]
                extras = list(extras_by_pass[pi])
                exi = [0]
                npoints = [14]
                # early passes run while their extras' E blocks are still in
                # flight: weave extras only into the back of the pass so the
                # in-order vector stream never blocks on a pending DMA.
                active_from = {0: 7, 1: 10}.get(pi, 14)
                def pull():
                    npoints[0] -= 1
                    if npoints[0] >= active_from:
                        return
                    rem = len(extras) - exi[0]
                    if rem <= 0:
                        return
                    n = -(-rem // max(npoints[0] + 1, 1))
                    for _ in range(n):
                        if exi[0] < len(extras):
                            extras[exi[0]]()
                            exi[0] += 1
                defA = deferred.get("A", []); defB = deferred.get("B", [])
                defC = deferred.get("C", [])

                def mmt():
                    return pp.tile([128, 512], F32, tag="mm", bufs=4,
                                   name="mmps")

                # --- z transform ---
                zt = []
                for h in range(2):
                    zps = mmt()
                    T.matmul(zps[:, :W], wgcn[:, h * 128:(h + 1) * 128],
                             agg[:, s0:s0 + W], start=True, stop=True)
                    z = zp.tile([128, 512], BF16, tag="z")
                    S.activation(z[:, :W], zps[:, :W], AF.Relu,
                                 bias=bgcn[:, h:h + 1])
                    zt.append(z)
                for f in defA:
                    f()
                pull()

                # --- L1 ---
                s1r = []
                for c in range(8):
                    lp = mmt()
                    T.matmul(lp[:, :W], w1[:, c * 128:c * 128 + 128],
                             zt[0][:, :W], start=True, stop=False)
                    T.matmul(lp[:, :W], w1[:, F1 + c * 128:F1 + c * 128 + 128],
                             zt[1][:, :W], start=False, stop=True)
                    sr = s1p.tile([128, 512], BF16, tag="s1")
                    if c in S1_ON_D:
                        D.tensor_scalar(sr[:, :W], lp[:, :W], b1c[:, c:c + 1],
                                        0.0, OP.add, OP.max)
                    else:
                        S.activation(sr[:, :W], lp[:, :W], AF.Relu,
                                     bias=b1c[:, c:c + 1])
                    s1r.append(sr)
                    pull()
                    if c == 3:
                        for f in defB:
                            f()
                    if c == 5:
                        for f in defC:
                            f()

                # --- M1 quadratic-form stats ---
                ds = []
                for h in range(2):
                    mzp = mmt()
                    for kk in range(2):
                        T.matmul(mzp[:, :W],
                                 m1[:, kk * HID + h * 128:kk * HID + h * 128 + 128],
                                 zt[kk][:, :W], start=(kk == 0), stop=(kk == 1))
                    dd = dp.tile([128, 512], BF16, tag="ds")
                    D.scalar_tensor_tensor(dd[:, :W], mzp[:, :W],
                                           cvec[:, h:h + 1], zt[h][:, :W],
                                           OP.add, OP.mult)
                    ds.append(dd)
                pull()

                # --- L2 + LN1/LN2 stats (v-space: u = v/std1, rstd1 absorbed) ---
                vts = []
                wt = std1b = None
                for c2 in range(4):
                    lp2 = mmt()
                    for k8 in range(8):
                        T.matmul(lp2[:, :W],
                                 w2[:, c2 * 1024 + k8 * 128:c2 * 1024 + k8 * 128 + 128],
                                 s1r[k8][:, :W], start=(k8 == 0), stop=(k8 == 7))
                    if c2 == 0:
                        ps1 = pp.tile([128, 512], F32, tag="stat", bufs=1)
                        for h in range(2):
                            T.matmul(ps1[:, :W], onesm[:], ds[h][:, :W],
                                     start=(h == 0), stop=(h == 1))
                        wt = vec.tile([128, 512], F32, tag="vec")
                        D.tensor_scalar(wt[:, :W], ps1[:, :W], EPS / F1,
                                        EPS * (EPS + c1const / F1),
                                        OP.mult, OP.add)
                        std1b = vec.tile([128, 512], F32, tag="vec")
                        S.activation(std1b[:, :W], wt[:, :W], AF.Sqrt,
                                     scale=1.0 / EPS)
                    v = uup.tile([128, 512], F32, tag="v")
                    D.scalar_tensor_tensor(v[:, :W], std1b[:, :W],
                                           b2c[:, c2:c2 + 1], lp2[:, :W],
                                           OP.mult, OP.add)
                    v2 = u2p.tile([128, 512], BF16, tag="v2")
                    S.activation(v2[:, :W], v[:, :W], AF.Square)
                    vts.append((v, v2))
                    pull()

                # --- LN2 stats + tail (c2 0,1 inline; 2,3 deferred) ---
                ps2 = pp.tile([128, 512], F32, tag="stat", bufs=1)
                for c2 in range(3):
                    T.matmul(ps2[:, :W], onesm[:], vts[c2][1][:, :W],
                             start=(c2 == 0), stop=False)
                pas = {}
                for c2 in range(2):
                    pa = pp.tile([128, 512], F32, tag="pa", bufs=2)
                    T.matmul(pa[:, :W], wa[:, c2 * 128:(c2 + 1) * 128],
                             actT[:, s0:s0 + W], start=True, stop=True)
                    pas[c2] = pa
                T.matmul(ps2[:, :W], onesm[:], vts[3][1][:, :W],
                         start=False, stop=True)
                var2t = vec.tile([128, 512], F32, tag="vec")
                D.scalar_tensor_tensor(var2t[:, :W], ps2[:, :W], 1.0 / F2,
                                       wt[:, :W], OP.mult, OP.add)
                std2 = vec.tile([128, 512], F32, tag="vec")
                S.activation(std2[:, :W], var2t[:, :W], AF.Sqrt)
                rstd2b = bcp.tile([128, 512], F32, tag="rstd")
                D.reciprocal_approx_fast(rstd2b[:, :W], std2[:, :W])
                wvs = {}
                for c2 in range(4):
                    wv = wvp.tile([128, 512], F32, tag="wv")
                    G.tensor_tensor(wv[:, :W], vts[c2][0][:, :W],
                                    rstd2b[:, :W], OP.mult)
                    wvs[c2] = wv
                sas = {}
                for c2 in range(2):
                    t2 = tlp.tile([128, 512], F32, tag="t2")
                    D.scalar_tensor_tensor(t2[:, :W], wvs[c2][:, :W],
                                           g2c[:, c2:c2 + 1], pas[c2][:, :W],
                                           OP.mult, OP.add)
                    sa = sap.tile([128, 512], BF16, tag="sa")
                    S.activation(sa[:, :W], t2[:, :W], AF.Relu)
                    sas[c2] = sa

                # --- deferred tail: wa/t2/sa for c2 2,3 + wq + q out ---
                def tail_A():
                    for c2 in (2, 3):
                        pa = pp.tile([128, 512], F32, tag="pa", bufs=2)
                        T.matmul(pa[:, :W], wa[:, c2 * 128:(c2 + 1) * 128],
                                 actT[:, s0:s0 + W], start=True, stop=True)
                        pas[c2] = pa
                    for c2 in (2, 3):
                        t2 = tlp.tile([128, 512], F32, tag="t2")
                        D.scalar_tensor_tensor(t2[:, :W], wvs[c2][:, :W],
                                               g2c[:, c2:c2 + 1],
                                               pas[c2][:, :W], OP.mult, OP.add)
                        sa = sap.tile([128, 512], BF16, tag="sa")
                        S.activation(sa[:, :W], t2[:, :W], AF.Relu)
                        sas[c2] = sa
                qps = {}
                def tail_B():
                    qp = pp.tile([1, 512], F32, tag="qp", bufs=1)
                    for c2 in range(4):
                        T.matmul(qp[:, :W], wq[:, c2:c2 + 1], sas[c2][:, :W],
                                 start=(c2 == 0), stop=(c2 == 3))
                    qps[0] = qp
                def tail_C():
                    qv = qvp.tile([1, 512], F32, tag="qv")
                    S.activation(qv[:, :W], qps[0][:, :W], AF.Copy, bias=bq)
                    SY.dma_start(OUT[:, s0:s0 + W], qv[:, :W])
                return {"A": [tail_A], "B": [tail_B], "C": [tail_C]}

            # ---------- emission ----------
            for f in agg_ops(0, (D,)):
                f()
            for f in agg_ops(1, (D,)):
                f()
            deferred = {}
            for pi in range(len(PASSES)):
                deferred = emit_pass(pi, deferred)
            for f in deferred["A"] + deferred["B"] + deferred["C"]:
                f()
    nc.compile()
    return nc


def kernel(**inputs):
    weights, percore, rows_list, meta = _preprocess(**inputs)

    key = (meta["Kb"], meta["tot_cols"])
    if key not in _KERNEL_CACHE:
        _KERNEL_CACHE[key] = _build(meta)
    nc = _KERNEL_CACHE[key]

    in_maps = []
    for c in range(N_CORES):
        m = dict(weights)
        m["E"] = percore["E"][c]
        m["actT"] = percore["actT"][c]
        in_maps.append(m)

    trace = os.environ.get("KERNEL_TRACE", "0") == "1"
    kw = {}
    if trace:
        import types, contextlib, ctypes
        if "antenv.axon_hooks" not in sys.modules:
            lib = ctypes.CDLL("/opt/axon/libaxon_pjrt.so")
            lib.axon_start_nrt_profile.argtypes = [
                ctypes.POINTER(ctypes.c_int64), ctypes.c_size_t]
            lib.axon_start_nrt_profile.restype = ctypes.c_int64
            lib.axon_stop_nrt_profile.argtypes = [ctypes.c_char_p]
            lib.axon_stop_nrt_profile.restype = ctypes.c_int64

            @contextlib.contextmanager
            def _hook(output_dir, device_ids):
                import jax
                jax.devices()
                if device_ids:
                    ids = (ctypes.c_int64 * len(device_ids))(*device_ids)
                    rc = lib.axon_start_nrt_profile(ids, len(device_ids))
                else:
                    rc = lib.axon_start_nrt_profile(None, 0)
                if rc != 0:
                    raise RuntimeError(f"axon_start_nrt_profile rc={rc}")
                try:
                    yield
                finally:
                    n = lib.axon_stop_nrt_profile(str(output_dir).encode())
                    print(f"profile: {n} file(s) written to {output_dir}",
                          file=sys.stderr)

            mod = types.ModuleType("antenv.axon_hooks")
            mod.get_axon_ntff_profile_hook = lambda: _hook
            sys.modules["antenv.axon_hooks"] = mod
        kw = dict(trace=True,
                  tmpdir=os.environ.get("KERNEL_TRACE_DIR") or None)

    res = run_bass_kernel_spmd(nc, in_maps, list(range(N_CORES)), **kw)
    if trace and res.exec_time_ns is not None:
        print(f"HW exec time: {res.exec_time_ns} ns")

    out = np.empty((N_AGENTS, 1), np.float32)
    for c in range(N_CORES):
        q = res.results[c]["q"].reshape(PA)
        out[rows_list[c], 0] = q
    return out


# revision 20
# speedup vs baseline: 1.0310x; 1.0310x over previous
"""Trainium2 Bass kernel for nn_CriticNetwork (GCN message passing + critic MLP).

Strategy (8 NeuronCores, SPMD, no collectives):
  - Only agg[agent_idx] rows are consumed downstream, so message passing is
    pruned to edges whose destination is an agent node, and the GCN transform
    is moved after aggregation: A_hat @ (x W) == (A_hat @ x) W.
  - Agents are globally sorted by indegree and dealt round-robin to the 8
    cores, so every core sees an identical degree profile. The host
    materializes each core's (dinv-prescaled, bf16) edge-source rows
    feature-major into a dense slot tensor E with a per-256-agent-block slot
    count K (tight padding), streamed in with large sequential DMAs.
  - Aggregation runs as bf16 pairwise add trees on the vector engine (2x DVE
    mode) with a minority share on gpsimd (no PSUM port, 0.42 add efficiency).
  - Head (critic MLP) runs feature-major with bf16 matmul operands (f32 PSUM
    accumulation). LayerNorm mean-centering is folded into W1/W2 host-side;
    LN1's sum-of-squares comes from the quadratic form z^T(W1f W1f^T)z +
    2(W1f b1c)^T z + const. LN1's rstd is absorbed into LN2 EXACTLY:
    with v := lp + b2c*std1 (u = v/std1), t = u*rstd2_ref = v/sqrt(
    mean_f v^2 + eps*var1til), so rstd1 is never materialized.
  - Emission is software-pipelined: group 0 is processed as two half-width
    passes so the PE starts ~4us in (p-state ramp), each pass's tail
    (wa/wq/t2/sa) is deferred into the next pass's matmul stream so the
    tensor engine never idles at group boundaries.
"""
import os
import sys

sys.path.insert(0, "/opt/trn_rl_repo")

import numpy as np
import ml_dtypes

import concourse.bass as bass
import concourse.tile as tile
import concourse.mybir as mybir
from concourse import bacc
from concourse.bass_utils import run_bass_kernel_spmd

# ---- problem constants (hardcoded per spec) ----
N_NODES = 50000
DIM = 128          # IN_DIM
HID = 256
F1 = 1024
F2 = 512
NACT = 64
N_EDGES = 800000
N_AGENTS = 16384
N_CORES = 8
PA = N_AGENTS // N_CORES      # 2048 agents per core
BLK = 256                     # slot-count granularity (agents per K-block)
NB = PA // BLK                # 8 K-blocks per core
EPS = 1e-5
# head passes: (col_start, width, [K-block ids]) -- first and last groups are
# split into half-width passes: the first so the tensor engine starts early
# during DMA warmup, the last so little compute remains after the final
# (largest) E block lands.
PASSES = [(0, 256, (0,)), (256, 256, (1,)), (512, 512, (2, 3)),
          (1024, 512, (4, 5)), (1536, 256, (6,)), (1792, 256, (7,))]
S1_ON_D = (5, 6)              # L1-relu feature tiles drained on vector engine

F32 = mybir.dt.float32
F32R = mybir.dt.float32r
BF16 = mybir.dt.bfloat16
AF = mybir.ActivationFunctionType
OP = mybir.AluOpType

_KERNEL_CACHE = {}


def _preprocess(x, action, W_gcn, b_gcn, W1, b1, g1, beta1, W2, b2, g2, beta2,
                Wa, ba, Wq, bq, edge_index, agent_idx):
    f32 = np.float32
    x = np.asarray(x, f32); action = np.asarray(action, f32)
    edge_index = np.asarray(edge_index); agent_idx = np.asarray(agent_idx)
    W_gcn = np.asarray(W_gcn, f32); b_gcn = np.asarray(b_gcn, f32)
    W1 = np.asarray(W1, f32); b1 = np.asarray(b1, f32)
    g1 = np.asarray(g1, f32); beta1 = np.asarray(beta1, f32)
    W2 = np.asarray(W2, f32); b2 = np.asarray(b2, f32)
    g2 = np.asarray(g2, f32); beta2 = np.asarray(beta2, f32)
    Wa = np.asarray(Wa, f32); ba = np.asarray(ba, f32)
    Wq = np.asarray(Wq, f32); bq = np.asarray(bq, f32)

    assert np.all(beta1 == 0.0) and np.all(g1 > 0.0), \
        "kernel fast path requires beta1==0 and g1>0 (module init guarantees this)"

    N = N_NODES
    loops = np.arange(N, dtype=edge_index.dtype)
    src_all = np.concatenate([edge_index[0], loops])
    dst_all = np.concatenate([edge_index[1], loops])
    deg = np.bincount(dst_all, minlength=N).astype(np.int64)
    dinv = (1.0 / np.sqrt(np.maximum(deg, 1.0))).astype(f32)

    order = np.argsort(dst_all, kind="stable")
    src_sorted = src_all[order]
    starts = np.searchsorted(dst_all[order], np.arange(N + 1))

    # global indegree sort + round-robin deal: rank r -> core r%8, pos r//8.
    ind_all = deg[agent_idx]
    rank = np.argsort(ind_all, kind="stable")
    # shared per-block K (identical across cores by construction)
    Kb = []
    for b in range(NB):
        mx = int(ind_all[rank[8 * BLK * b: 8 * BLK * (b + 1)]].max())
        Kb.append(max(2, ((mx + 1) // 2) * 2))
    boff = np.concatenate([[0], np.cumsum(Kb)]).astype(int)
    tot_cols = int(boff[-1]) * BLK

    # prescaled node features, plus a zero pad row for empty slots
    xsf = np.zeros((N + 1, DIM), f32)
    xsf[:N] = x * dinv[:, None]

    E_list, actT_list, rows_list = [], [], []
    for c in range(N_CORES):
        rows = rank[np.arange(PA) * 8 + c]          # original agent rows
        ag = agent_idx[rows]
        dd = dinv[ag]
        Ec = np.empty((128, tot_cols), ml_dtypes.bfloat16)
        for b in range(NB):
            K = Kb[b]
            tbl = np.full((K, BLK), N, np.int64)
            for j in range(BLK):
                a = int(ag[b * BLK + j]); d = int(deg[a]); s = starts[a]
                tbl[:d, j] = src_sorted[s:s + d]
            blk = (xsf[tbl] * dd[b * BLK:(b + 1) * BLK][None, :, None])
            Ec[:, boff[b] * BLK:(boff[b] + K) * BLK] = (
                blk.transpose(2, 0, 1).reshape(128, K * BLK)
            ).astype(ml_dtypes.bfloat16)
        E_list.append(Ec)
        actp = action[rows].T                        # [64, PA]
        actT_list.append(np.ascontiguousarray(np.concatenate(
            [actp, np.ones((1, PA), f32)], axis=0)).astype(ml_dtypes.bfloat16))
        rows_list.append(rows)

    # ---- weight folding (exact algebra) ----
    w1m = W1.mean(axis=1)
    W1f = W1 - w1m[:, None]
    b1c = b1 - b1.mean()
    W2g = g1[:, None] * W2
    w2gm = W2g.mean(axis=1)
    W2f = W2g - w2gm[:, None]
    b2c = b2 - b2.mean()
    bb = ba + beta2
    M1 = (W1f @ W1f.T).astype(f32)
    cvec = (2.0 * (W1f @ b1c)).astype(f32)
    c1const = float(np.sum(b1c * b1c))

    def ktile_pack(W, kt, fdim):   # [kt*128, fdim] -> [128, kt*fdim]
        return np.ascontiguousarray(
            W.reshape(kt, 128, fdim).transpose(1, 0, 2).reshape(128, kt * fdim))

    # w2 packed c2-major: [128, c2*1024 + k8*128] so the DMA can stream the
    # c2=0 stationaries first (L2 of pass 0 starts before the full load).
    w2p = ktile_pack(W2f, 8, F2).reshape(128, 8, 4, 128).transpose(
        0, 2, 1, 3).reshape(128, 8 * F2)

    bf = ml_dtypes.bfloat16
    weights = {
        "wgcn": W_gcn.astype(bf),                               # [128, 256]
        "w1": ktile_pack(W1f, 2, F1).astype(bf),                # [128, 2048]
        "w2": np.ascontiguousarray(w2p).astype(bf),             # [128, 4096]
        "m1": ktile_pack(M1, 2, HID).astype(bf),                # [128, 512]
        "wa": np.ascontiguousarray(
            np.concatenate([Wa, bb[None, :]], axis=0)).astype(bf),  # [65, 512]
        "wq": np.ascontiguousarray(Wq.reshape(4, 128).T).astype(bf),  # [128, 4]
        "cols": np.ascontiguousarray(np.concatenate([
            b_gcn.reshape(2, 128).T,      # [:, 0:2]   bgcn
            b1c.reshape(8, 128).T,        # [:, 2:10]  b1c
            cvec.reshape(2, 128).T,       # [:, 10:12] cvec
            b2c.reshape(4, 128).T,        # [:, 12:16] b2c
            g2.reshape(4, 128).T,         # [:, 16:20] g2
        ], axis=1).astype(f32)),
        "onesmat_in": np.ones((128, 128), bf),
    }
    meta = dict(Kb=tuple(int(k) for k in Kb),
                boff=tuple(int(o) for o in boff),
                tot_cols=tot_cols, bq=float(bq[0]), c1const=c1const)
    percore = dict(E=E_list, actT=actT_list)
    return weights, percore, rows_list, meta


def _build(meta):
    Kb = meta["Kb"]; boff = meta["boff"]
    tot_cols = meta["tot_cols"]; bq = meta["bq"]; c1const = meta["c1const"]

    nc = bacc.Bacc("TRN2", target_bir_lowering=False, debug=False,
                   num_devices=N_CORES, num_swdge_queues=4)
    dram = {}
    def din(name, shape, dt):
        dram[name] = nc.dram_tensor(name, shape, dt, kind="ExternalInput").ap()
        return dram[name]

    E_d = din("E", [128, tot_cols], BF16)
    actT_d = din("actT", [NACT + 1, PA], BF16)
    wgcn_d = din("wgcn", [128, HID], BF16)
    w1_d = din("w1", [128, 2 * F1], BF16)
    w2_d = din("w2", [128, 8 * F2], BF16)
    m1_d = din("m1", [128, 2 * HID], BF16)
    wa_d = din("wa", [NACT + 1, F2], BF16)
    wq_d = din("wq", [128, 4], BF16)
    cols_d = din("cols", [128, 20], F32)
    onesm_d = din("onesmat_in", [128, 128], BF16)
    OUT = nc.dram_tensor("q", [1, PA], F32, kind="ExternalOutput").ap()

    with tile.TileContext(nc) as tc:
        with tc.tile_pool(name="w", bufs=1) as wp, \
             tc.tile_pool(name="eb", bufs=1) as ep, \
             tc.tile_pool(name="prp", bufs=3) as prp, \
             tc.tile_pool(name="csp", bufs=12) as csp, \
             tc.tile_pool(name="zp", bufs=3) as zp, \
             tc.tile_pool(name="s1p", bufs=9) as s1p, \
             tc.tile_pool(name="dp", bufs=3) as dp, \
             tc.tile_pool(name="uup", bufs=5) as uup, \
             tc.tile_pool(name="u2p", bufs=3) as u2p, \
             tc.tile_pool(name="vec", bufs=6) as vec, \
             tc.tile_pool(name="bcp", bufs=3) as bcp, \
             tc.tile_pool(name="tlp", bufs=4) as tlp, \
             tc.tile_pool(name="wvp", bufs=4) as wvp, \
             tc.tile_pool(name="sap", bufs=8) as sap, \
             tc.tile_pool(name="qvp", bufs=2) as qvp, \
             tc.tile_pool(name="ps", bufs=1, space="PSUM") as pp:

            T = nc.tensor; S = nc.scalar; D = nc.vector; G = nc.gpsimd
            SY = nc.sync

            # ---------- DMA: E blocks resident, spread across queues ----------
            # early phase is DMA-bandwidth-bound: each E block is half-split
            # across the sync + scalar queues in strict consumption order, so
            # block b lands as early as the aggregate bandwidth allows.
            # weights ride the gpsimd queue in first-use order.
            eb = [ep.tile([128, Kb[b] * BLK], BF16, name=f"eb{b}")
                  for b in range(NB)]
            def eslice(b, lo, hi):
                return (eb[b][:, lo * BLK:hi * BLK],
                        E_d[:, (boff[b] + lo) * BLK:(boff[b] + hi) * BLK])
            def ehalves(b):
                K = Kb[b]
                h1 = min(max(4, (K // 2 + 3) // 4 * 4), K)
                return h1, K
            # all tiles first (names), then the interleaved issue schedule
            wgcn = wp.tile([128, HID], BF16)
            cols = wp.tile([128, 20], F32)
            actT = wp.tile([NACT + 1, PA], BF16)
            wa = wp.tile([NACT + 1, F2], BF16)
            wq = wp.tile([128, 4], BF16)
            w1 = wp.tile([128, 2 * F1], BF16)
            m1 = wp.tile([128, 2 * HID], BF16)
            onesm = wp.tile([128, 128], BF16)
            w2 = wp.tile([128, 8 * F2], BF16)
            def w2c(c2):
                return (w2[:, c2 * 1024:(c2 + 1) * 1024],
                        w2_d[:, c2 * 1024:(c2 + 1) * 1024])
            # sync queue: b0 first, then weights interleaved with E halves
            SY.dma_start(wgcn[:], wgcn_d[:])
            SY.dma_start(*eslice(0, 0, 4))
            SY.dma_start(*eslice(0, 4, ehalves(0)[0]))
            SY.dma_start(*eslice(1, 0, ehalves(1)[0]))
            SY.dma_start(w1[:], w1_d[:])
            SY.dma_start(*eslice(2, 0, ehalves(2)[0]))
            SY.dma_start(*w2c(0))
            SY.dma_start(*w2c(2))
            SY.dma_start(*eslice(3, 0, ehalves(3)[0]))
            SY.dma_start(*eslice(4, 0, ehalves(4)[0]))
            SY.dma_start(*eslice(5, 0, ehalves(5)[0]))
            SY.dma_start(*eslice(6, 0, ehalves(6)[0]))
            # scalar queue: small early weights + E second halves
            S.dma_start(cols[:], cols_d[:])
            S.dma_start(*eslice(0, ehalves(0)[0], Kb[0]))
            S.dma_start(*eslice(1, ehalves(1)[0], Kb[1]))
            S.dma_start(m1[:], m1_d[:])
            S.dma_start(onesm[:], onesm_d[:])
            S.dma_start(actT[:], actT_d[:])
            S.dma_start(wa[:], wa_d[:])
            S.dma_start(*eslice(2, ehalves(2)[0], Kb[2]))
            S.dma_start(*w2c(1))
            S.dma_start(*w2c(3))
            S.dma_start(*eslice(3, ehalves(3)[0], Kb[3]))
            S.dma_start(*eslice(4, ehalves(4)[0], Kb[4]))
            S.dma_start(*eslice(6, ehalves(6)[0], Kb[6]))
            S.dma_start(*eslice(7, ehalves(7)[0], Kb[7]))
            # gpsimd queue (slow): tiny wq + late E halves for extra bandwidth
            G.dma_start(wq[:], wq_d[:])
            G.dma_start(*eslice(5, ehalves(5)[0], Kb[5]))
            G.dma_start(*eslice(7, 0, ehalves(7)[0]))

            bgcn = cols[:, 0:2]
            b1c = cols[:, 2:10]
            cvec = cols[:, 10:12]
            b2c = cols[:, 12:16]
            g2c = cols[:, 16:20]
            agg = wp.tile([128, PA], BF16)       # agg^T, feature-major

            # ---------- aggregation: bf16 pairwise trees, D majority ----------
            def agg_ops(b, pattern):
                """Closures (dependency-ordered) summing E block b -> agg."""
                K = Kb[b]; et = eb[b]
                av = agg[:, b * BLK:(b + 1) * BLK]
                ops = []
                sums = {"D": [], "G": []}
                nch = K // 4
                for q in range(nch):
                    eng = pattern[q % len(pattern)]
                    nm = "D" if eng is D else "G"
                    c3 = et[:, 4 * q * BLK:(4 * q + 4) * BLK].rearrange(
                        "p (s e) -> p s e", e=BLK)
                    pr = prp.tile([128, 2 * BLK], BF16, tag="pr" + nm)
                    p3 = pr[:].rearrange("p (s e) -> p s e", e=BLK)
                    ops.append(lambda eng=eng, p3=p3, c3=c3: eng.tensor_tensor(
                        p3, c3[:, 0::2, :], c3[:, 1::2, :], OP.add))
                    cs = csp.tile([128, BLK], BF16, tag="cs" + nm)
                    ops.append(lambda eng=eng, cs=cs, pr=pr: eng.tensor_tensor(
                        cs[:], pr[:, :BLK], pr[:, BLK:], OP.add))
                    sums[nm].append(cs)
                if K - 4 * nch == 2:
                    eng = pattern[nch % len(pattern)]
                    nm = "D" if eng is D else "G"
                    c3 = et[:, 4 * nch * BLK:(4 * nch + 2) * BLK].rearrange(
                        "p (s e) -> p s e", e=BLK)
                    cs = csp.tile([128, BLK], BF16, tag="cs" + nm)
                    ops.append(lambda eng=eng, cs=cs, c3=c3: eng.tensor_tensor(
                        cs[:], c3[:, 0, :], c3[:, 1, :], OP.add))
                    sums[nm].append(cs)
                parts = []
                for nm, eng in (("D", D), ("G", G)):
                    lst = sums[nm]
                    while len(lst) > 1:
                        nxt = []
                        for i in range(0, len(lst) - 1, 2):
                            t = csp.tile([128, BLK], BF16, tag="cs" + nm)
                            ops.append(lambda eng=eng, t=t, a=lst[i],
                                       b2=lst[i + 1]: eng.tensor_tensor(
                                           t[:], a[:], b2[:], OP.add))
                            nxt.append(t)
                        if len(lst) % 2:
                            nxt.append(lst[-1])
                        lst = nxt
                    if lst:
                        parts.append(lst[0])
                if len(parts) == 2:
                    ops.append(lambda a=parts[0], b2=parts[1]: D.tensor_tensor(
                        av, a[:], b2[:], OP.add))
                else:
                    ops.append(lambda a=parts[0]: D.tensor_scalar(
                        av, a[:], 0.0, None, OP.add))
                return ops

            extras_by_pass = {
                0: agg_ops(2, (D, D, D, G)),
                1: agg_ops(3, (D, D, D, G)),
                2: agg_ops(4, (D, D, D, G)) + agg_ops(5, (D, D, D, G)),
                3: agg_ops(6, (D, D, D, G)) + agg_ops(7, (D, D, D, G)),
                4: [],
                5: [],
            }

            # ---------- head pass emitter (software pipelined) ----------
            def emit_pass(pi, deferred):
                s0, W, _ = PASSES[pi]
                extras = list(extras_by_pass[pi])
                exi = [0]
                npoints = [14]
                # early passes run while their extras' E blocks are still in
                # flight: weave extras only into the back of the pass so the
                # in-order vector stream never blocks on a pending DMA.
                active_from = {0: 7, 1: 10}.get(pi, 14)
                def pull():
                    npoints[0] -= 1
                    if npoints[0] >= active_from:
                        return
                    rem = len(extras) - exi[0]
                    if rem <= 0:
                        return
                    n = -(-rem // max(npoints[0] + 1, 1))
                    for _ in range(n):
                        if exi[0] < len(extras):
                            extras[exi[0]]()
                            exi[0] += 1
                defA = deferred.get("A", []); defB = deferred.get("B", [])
                defC = deferred.get("C", [])

                def mmt():
                    return pp.tile([128, 512], F32, tag="mm", bufs=4,
                                   name="mmps")

                # --- z transform ---
                zt = []
                for h in range(2):
                    zps = mmt()
                    T.matmul(zps[:, :W], wgcn[:, h * 128:(h + 1) * 128],
                             agg[:, s0:s0 + W], start=True, stop=True)
                    z = zp.tile([128, 512], BF16, tag="z")
                    S.activation(z[:, :W], zps[:, :W], AF.Relu,
                                 bias=bgcn[:, h:h + 1])
                    zt.append(z)
                for f in defA:
                    f()
                pull()

                # --- L1 ---
                s1r = []
                for c in range(8):
                    lp = mmt()
                    T.matmul(lp[:, :W], w1[:, c * 128:c * 128 + 128],
                             zt[0][:, :W], start=True, stop=False)
                    T.matmul(lp[:, :W], w1[:, F1 + c * 128:F1 + c * 128 + 128],
                             zt[1][:, :W], start=False, stop=True)
                    sr = s1p.tile([128, 512], BF16, tag="s1")
                    if c in S1_ON_D:
                        D.tensor_scalar(sr[:, :W], lp[:, :W], b1c[:, c:c + 1],
                                        0.0, OP.add, OP.max)
                    else:
                        S.activation(sr[:, :W], lp[:, :W], AF.Relu,
                                     bias=b1c[:, c:c + 1])
                    s1r.append(sr)
                    pull()
                    if c == 3:
                        for f in defB:
                            f()
                    if c == 5:
                        for f in defC:
                            f()

                # --- M1 quadratic-form stats ---
                ds = []
                for h in range(2):
                    mzp = mmt()
                    for kk in range(2):
                        T.matmul(mzp[:, :W],
                                 m1[:, kk * HID + h * 128:kk * HID + h * 128 + 128],
                                 zt[kk][:, :W], start=(kk == 0), stop=(kk == 1))
                    dd = dp.tile([128, 512], BF16, tag="ds")
                    D.scalar_tensor_tensor(dd[:, :W], mzp[:, :W],
                                           cvec[:, h:h + 1], zt[h][:, :W],
                                           OP.add, OP.mult)
                    ds.append(dd)
                dsum = dp.tile([128, 512], BF16, tag="ds")
                G.tensor_tensor(dsum[:, :W], ds[0][:, :W], ds[1][:, :W], OP.add)
                pull()

                # --- L2 + LN1/LN2 stats (v-space: u = v/std1, rstd1 absorbed) ---
                vts = []
                wt = std1b = None
                for c2 in range(4):
                    lp2 = mmt()
                    for k8 in range(8):
                        T.matmul(lp2[:, :W],
                                 w2[:, c2 * 1024 + k8 * 128:c2 * 1024 + k8 * 128 + 128],
                                 s1r[k8][:, :W], start=(k8 == 0), stop=(k8 == 7))
                    if c2 == 0:
                        ps1 = pp.tile([128, 512], F32, tag="stat", bufs=1)
                        T.matmul(ps1[:, :W], onesm[:], dsum[:, :W],
                                 start=True, stop=True)
                        wt = vec.tile([128, 512], F32, tag="vec")
                        D.tensor_scalar(wt[:, :W], ps1[:, :W], EPS / F1,
                                        EPS * (EPS + c1const / F1),
                                        OP.mult, OP.add)
                        std1b = vec.tile([128, 512], F32, tag="vec")
                        S.activation(std1b[:, :W], wt[:, :W], AF.Sqrt,
                                     scale=1.0 / EPS)
                    v = uup.tile([128, 512], F32, tag="v")
                    D.scalar_tensor_tensor(v[:, :W], std1b[:, :W],
                                           b2c[:, c2:c2 + 1], lp2[:, :W],
                                           OP.mult, OP.add)
                    v2 = u2p.tile([128, 512], BF16, tag="v2")
                    S.activation(v2[:, :W], v[:, :W], AF.Square)
                    vts.append((v, v2))
                    pull()

                # --- LN2 stats + tail (c2 0,1 inline; 2,3 deferred) ---
                # v2 tiles pre-summed on gpsimd so the stat is 1 matmul not 4
                v2a = u2p.tile([128, 512], BF16, tag="v2s")
                G.tensor_tensor(v2a[:, :W], vts[0][1][:, :W], vts[1][1][:, :W],
                                OP.add)
                v2b = u2p.tile([128, 512], BF16, tag="v2s")
                G.tensor_tensor(v2b[:, :W], vts[2][1][:, :W], vts[3][1][:, :W],
                                OP.add)
                v2s = u2p.tile([128, 512], BF16, tag="v2s")
                G.tensor_tensor(v2s[:, :W], v2a[:, :W], v2b[:, :W], OP.add)
                pas = {}
                for c2 in range(2):
                    pa = pp.tile([128, 512], F32, tag="pa", bufs=2)
                    T.matmul(pa[:, :W], wa[:, c2 * 128:(c2 + 1) * 128],
                             actT[:, s0:s0 + W], start=True, stop=True)
                    pas[c2] = pa
                ps2 = pp.tile([128, 512], F32, tag="stat", bufs=1)
                T.matmul(ps2[:, :W], onesm[:], v2s[:, :W],
                         start=True, stop=True)
                var2t = vec.tile([128, 512], F32, tag="vec")
                D.scalar_tensor_tensor(var2t[:, :W], ps2[:, :W], 1.0 / F2,
                                       wt[:, :W], OP.mult, OP.add)
                std2 = vec.tile([128, 512], F32, tag="vec")
                S.activation(std2[:, :W], var2t[:, :W], AF.Sqrt)
                rstd2b = bcp.tile([128, 512], F32, tag="rstd")
                D.reciprocal_approx_fast(rstd2b[:, :W], std2[:, :W])
                wvs = {}
                for c2 in range(4):
                    wv = wvp.tile([128, 512], F32, tag="wv")
                    G.tensor_tensor(wv[:, :W], vts[c2][0][:, :W],
                                    rstd2b[:, :W], OP.mult)
                    wvs[c2] = wv
                sas = {}
                for c2 in range(2):
                    t2 = tlp.tile([128, 512], F32, tag="t2")
                    D.scalar_tensor_tensor(t2[:, :W], wvs[c2][:, :W],
                                           g2c[:, c2:c2 + 1], pas[c2][:, :W],
                                           OP.mult, OP.add)
                    sa = sap.tile([128, 512], BF16, tag="sa")
                    S.activation(sa[:, :W], t2[:, :W], AF.Relu)
                    sas[c2] = sa

                # --- deferred tail: wa/t2/sa for c2 2,3 + wq + q out ---
                def tail_A():
                    for c2 in (2, 3):
                        pa = pp.tile([128, 512], F32, tag="pa", bufs=2)
                        T.matmul(pa[:, :W], wa[:, c2 * 128:(c2 + 1) * 128],
                                 actT[:, s0:s0 + W], start=True, stop=True)
                        pas[c2] = pa
                    for c2 in (2, 3):
                        t2 = tlp.tile([128, 512], F32, tag="t2")
                        D.scalar_tensor_tensor(t2[:, :W], wvs[c2][:, :W],
                                               g2c[:, c2:c2 + 1],
                                               pas[c2][:, :W], OP.mult, OP.add)
                        sa = sap.tile([128, 512], BF16, tag="sa")
                        S.activation(sa[:, :W], t2[:, :W], AF.Relu)
                        sas[c2] = sa
                qps = {}
                def tail_B():
                    qp = pp.tile([1, 512], F32, tag="qp", bufs=1)
                    for c2 in range(4):
                        T.matmul(qp[:, :W], wq[:, c2:c2 + 1], sas[c2][:, :W],
                                 start=(c2 == 0), stop=(c2 == 3))
                    qps[0] = qp
                def tail_C():
                    qv = qvp.tile([1, 512], F32, tag="qv")
                    S.activation(qv[:, :W], qps[0][:, :W], AF.Copy, bias=bq)
                    SY.dma_start(OUT[:, s0:s0 + W], qv[:, :W])
                return {"A": [tail_A], "B": [tail_B], "C": [tail_C]}

            # ---------- emission ----------
            for f in agg_ops(0, (D,)):
                f()
            for f in agg_ops(1, (D,)):
                f()
            deferred = {}
            for pi in range(len(PASSES)):
                deferred = emit_pass(pi, deferred)
            for f in deferred["A"] + deferred["B"] + deferred["C"]:
                f()
    nc.compile()
    return nc


def kernel(**inputs):
    weights, percore, rows_list, meta = _preprocess(**inputs)

    key = (meta["Kb"], meta["tot_cols"])
    if key not in _KERNEL_CACHE:
        _KERNEL_CACHE[key] = _build(meta)
    nc = _KERNEL_CACHE[key]

    in_maps = []
    for c in range(N_CORES):
        m = dict(weights)
        m["E"] = percore["E"][c]
        m["actT"] = percore["actT"][c]
        in_maps.append(m)

    trace = os.environ.get("KERNEL_TRACE", "0") == "1"
    kw = {}
    if trace:
        import types, contextlib, ctypes
        if "antenv.axon_hooks" not in sys.modules:
            lib = ctypes.CDLL("/opt/axon/libaxon_pjrt.so")
            lib.axon_start_nrt_profile.argtypes = [
                ctypes.POINTER(ctypes.c_int64), ctypes.c_size_t]
            lib.axon_start_nrt_profile.restype = ctypes.c_int64
            lib.axon_stop_nrt_profile.argtypes = [ctypes.c_char_p]
            lib.axon_stop_nrt_profile.restype = ctypes.c_int64

            @contextlib.contextmanager
            def _hook(output_dir, device_ids):
                import jax
                jax.devices()
                if device_ids:
                    ids = (ctypes.c_int64 * len(device_ids))(*device_ids)
                    rc = lib.axon_start_nrt_profile(ids, len(device_ids))
                else:
                    rc = lib.axon_start_nrt_profile(None, 0)
                if rc != 0:
                    raise RuntimeError(f"axon_start_nrt_profile rc={rc}")
                try:
                    yield
                finally:
                    n = lib.axon_stop_nrt_profile(str(output_dir).encode())
                    print(f"profile: {n} file(s) written to {output_dir}",
                          file=sys.stderr)

            mod = types.ModuleType("antenv.axon_hooks")
            mod.get_axon_ntff_profile_hook = lambda: _hook
            sys.modules["antenv.axon_hooks"] = mod
        kw = dict(trace=True,
                  tmpdir=os.environ.get("KERNEL_TRACE_DIR") or None)

    res = run_bass_kernel_spmd(nc, in_maps, list(range(N_CORES)), **kw)
    if trace and res.exec_time_ns is not None:
        print(f"HW exec time: {res.exec_time_ns} ns")

    out = np.empty((N_AGENTS, 1), np.float32)
    for c in range(N_CORES):
        q = res.results[c]["q"].reshape(PA)
        out[rows_list[c], 0] = q
    return out


# revision 58
# speedup vs baseline: 1.3028x; 1.2637x over previous
"""Trainium2 Bass kernel for nn_CriticNetwork (GCN message passing + critic MLP).

Strategy (8 NeuronCores, SPMD, no collectives):
  - Only agg[agent_idx] rows are consumed downstream, so message passing is
    pruned to edges whose destination is an agent node, and the GCN transform
    is moved after aggregation: A_hat @ (x W) == (A_hat @ x) W.
  - Agents are globally sorted by indegree and dealt round-robin to the 8
    cores, so every core sees an identical degree profile. The host
    materializes each core's (dinv-prescaled, bf16) edge-source rows
    feature-major into a dense slot tensor E with a per-256-agent-block slot
    count K (tight padding), streamed in with large sequential DMAs.
  - Aggregation runs as bf16 pairwise add trees on the vector engine (2x DVE
    mode) with a minority share on gpsimd (no PSUM port, 0.42 add efficiency).
  - Head (critic MLP) runs feature-major with bf16 matmul operands (f32 PSUM
    accumulation). LayerNorm mean-centering is folded into W1/W2 host-side;
    LN1's sum-of-squares comes from the quadratic form z^T(W1f W1f^T)z +
    2(W1f b1c)^T z + const. LN1's rstd is absorbed into LN2 EXACTLY:
    with v := lp + b2c*std1 (u = v/std1), t = u*rstd2_ref = v/sqrt(
    mean_f v^2 + eps*var1til), so rstd1 is never materialized.
  - Emission is software-pipelined: the first and last agent groups run as
    half-width passes (early PE start during DMA warmup; little compute left
    after the last E block lands), each pass's z-transform is hoisted into
    the previous pass, tails (wa/wq) are deferred into the next pass's
    matmul stream, and E rides the sync+scalar DMA queues (pure E, in
    consumption order, half-split per block) with weights on the gpsimd
    queue -- the early phase is DMA-bandwidth-bound at ~115 GB/s per queue.
  - PSUM: 5-bank matmul pool + 1 action + 1 stat + 1 q; all L1 drains on the
    scalar engine (gpsimd has no PSUM port; vector is the congested engine).
"""
import os
import sys

sys.path.insert(0, "/opt/trn_rl_repo")

import numpy as np
import ml_dtypes

import concourse.bass as bass
import concourse.tile as tile
import concourse.mybir as mybir
from concourse import bacc
from concourse.bass_utils import run_bass_kernel_spmd

# ---- problem constants (hardcoded per spec) ----
N_NODES = 50000
DIM = 128          # IN_DIM
HID = 256
F1 = 1024
F2 = 512
NACT = 64
N_EDGES = 800000
N_AGENTS = 16384
N_CORES = 8
PA = N_AGENTS // N_CORES      # 2048 agents per core
BLK = 256                     # slot-count granularity (agents per K-block)
NB = PA // BLK                # 8 K-blocks per core
EPS = 1e-5
# head passes: (col_start, width, [K-block ids]) -- first and last groups are
# split into half-width passes: the first so the tensor engine starts early
# during DMA warmup, the last so little compute remains after the final
# (largest) E block lands.
PASSES = [(0, 256, (0,)), (256, 256, (1,)), (512, 512, (2, 3)),
          (1024, 512, (4, 5)), (1536, 256, (6,)), (1792, 256, (7,))]
S1_ON_D = ()                  # L1-relu feature tiles drained on vector engine

F32 = mybir.dt.float32
F32R = mybir.dt.float32r
BF16 = mybir.dt.bfloat16
AF = mybir.ActivationFunctionType
OP = mybir.AluOpType

_KERNEL_CACHE = {}


def _preprocess(x, action, W_gcn, b_gcn, W1, b1, g1, beta1, W2, b2, g2, beta2,
                Wa, ba, Wq, bq, edge_index, agent_idx):
    f32 = np.float32
    x = np.asarray(x, f32); action = np.asarray(action, f32)
    edge_index = np.asarray(edge_index); agent_idx = np.asarray(agent_idx)
    W_gcn = np.asarray(W_gcn, f32); b_gcn = np.asarray(b_gcn, f32)
    W1 = np.asarray(W1, f32); b1 = np.asarray(b1, f32)
    g1 = np.asarray(g1, f32); beta1 = np.asarray(beta1, f32)
    W2 = np.asarray(W2, f32); b2 = np.asarray(b2, f32)
    g2 = np.asarray(g2, f32); beta2 = np.asarray(beta2, f32)
    Wa = np.asarray(Wa, f32); ba = np.asarray(ba, f32)
    Wq = np.asarray(Wq, f32); bq = np.asarray(bq, f32)

    assert np.all(beta1 == 0.0) and np.all(g1 > 0.0), \
        "kernel fast path requires beta1==0 and g1>0 (module init guarantees this)"

    N = N_NODES
    loops = np.arange(N, dtype=edge_index.dtype)
    src_all = np.concatenate([edge_index[0], loops])
    dst_all = np.concatenate([edge_index[1], loops])
    deg = np.bincount(dst_all, minlength=N).astype(np.int64)
    dinv = (1.0 / np.sqrt(np.maximum(deg, 1.0))).astype(f32)

    order = np.argsort(dst_all, kind="stable")
    src_sorted = src_all[order]
    starts = np.searchsorted(dst_all[order], np.arange(N + 1))

    # global indegree sort + round-robin deal: rank r -> core r%8, pos r//8.
    ind_all = deg[agent_idx]
    rank = np.argsort(ind_all, kind="stable")
    # shared per-block K (identical across cores by construction)
    Kb = []
    for b in range(NB):
        mx = int(ind_all[rank[8 * BLK * b: 8 * BLK * (b + 1)]].max())
        Kb.append(max(2, ((mx + 1) // 2) * 2))
    boff = np.concatenate([[0], np.cumsum(Kb)]).astype(int)
    tot_cols = int(boff[-1]) * BLK

    # prescaled node features, plus a zero pad row for empty slots
    xsf = np.zeros((N + 1, DIM), f32)
    xsf[:N] = x * dinv[:, None]

    E_list, actT_list, rows_list = [], [], []
    for c in range(N_CORES):
        rows = rank[np.arange(PA) * 8 + c]          # original agent rows
        ag = agent_idx[rows]
        dd = dinv[ag]
        Ec = np.empty((128, tot_cols), ml_dtypes.bfloat16)
        for b in range(NB):
            K = Kb[b]
            tbl = np.full((K, BLK), N, np.int64)
            for j in range(BLK):
                a = int(ag[b * BLK + j]); d = int(deg[a]); s = starts[a]
                tbl[:d, j] = src_sorted[s:s + d]
            blk = (xsf[tbl] * dd[b * BLK:(b + 1) * BLK][None, :, None])
            Ec[:, boff[b] * BLK:(boff[b] + K) * BLK] = (
                blk.transpose(2, 0, 1).reshape(128, K * BLK)
            ).astype(ml_dtypes.bfloat16)
        E_list.append(Ec)
        actp = action[rows].T                        # [64, PA]
        actT_list.append(np.ascontiguousarray(np.concatenate(
            [actp, np.ones((1, PA), f32)], axis=0)).astype(ml_dtypes.bfloat16))
        rows_list.append(rows)

    # ---- weight folding (exact algebra) ----
    w1m = W1.mean(axis=1)
    W1f = W1 - w1m[:, None]
    b1c = b1 - b1.mean()
    W2g = g1[:, None] * W2
    w2gm = W2g.mean(axis=1)
    W2f = W2g - w2gm[:, None]
    b2c = b2 - b2.mean()
    bb = ba + beta2
    M1 = (W1f @ W1f.T).astype(f32)
    cvec = (2.0 * (W1f @ b1c)).astype(f32)
    c1const = float(np.sum(b1c * b1c))

    def ktile_pack(W, kt, fdim):   # [kt*128, fdim] -> [128, kt*fdim]
        return np.ascontiguousarray(
            W.reshape(kt, 128, fdim).transpose(1, 0, 2).reshape(128, kt * fdim))

    # w2 packed c2-major: [128, c2*1024 + k8*128] so the DMA can stream the
    # c2=0 stationaries first (L2 of pass 0 starts before the full load).
    w2p = ktile_pack(W2f, 8, F2).reshape(128, 8, 4, 128).transpose(
        0, 2, 1, 3).reshape(128, 8 * F2)

    bf = ml_dtypes.bfloat16
    weights = {
        "wgcn": W_gcn.astype(bf),                               # [128, 256]
        "w1": ktile_pack(W1f, 2, F1).astype(bf),                # [128, 2048]
        "w2": np.ascontiguousarray(w2p).astype(bf),             # [128, 4096]
        "m1": ktile_pack(M1, 2, HID).astype(bf),                # [128, 512]
        "wa": np.ascontiguousarray(
            np.concatenate([Wa, bb[None, :]], axis=0)).astype(bf),  # [65, 512]
        "wq": np.ascontiguousarray(Wq.reshape(4, 128).T).astype(bf),  # [128, 4]
        "cols": np.ascontiguousarray(np.concatenate([
            b_gcn.reshape(2, 128).T,      # [:, 0:2]   bgcn
            b1c.reshape(8, 128).T,        # [:, 2:10]  b1c
            cvec.reshape(2, 128).T,       # [:, 10:12] cvec
            b2c.reshape(4, 128).T,        # [:, 12:16] b2c
            g2.reshape(4, 128).T,         # [:, 16:20] g2
        ], axis=1).astype(f32)),
        "onesmat_in": np.ones((128, 128), bf),
        "ident_in": np.eye(128, dtype=bf),
    }
    meta = dict(Kb=tuple(int(k) for k in Kb),
                boff=tuple(int(o) for o in boff),
                tot_cols=tot_cols, bq=float(bq[0]), c1const=c1const)
    percore = dict(E=E_list, actT=actT_list)
    return weights, percore, rows_list, meta


def _build(meta):
    Kb = meta["Kb"]; boff = meta["boff"]
    tot_cols = meta["tot_cols"]; bq = meta["bq"]; c1const = meta["c1const"]

    nc = bacc.Bacc("TRN2", target_bir_lowering=False, debug=False,
                   num_devices=N_CORES, num_swdge_queues=4)
    dram = {}
    def din(name, shape, dt):
        dram[name] = nc.dram_tensor(name, shape, dt, kind="ExternalInput").ap()
        return dram[name]

    E_d = din("E", [128, tot_cols], BF16)
    actT_d = din("actT", [NACT + 1, PA], BF16)
    wgcn_d = din("wgcn", [128, HID], BF16)
    w1_d = din("w1", [128, 2 * F1], BF16)
    w2_d = din("w2", [128, 8 * F2], BF16)
    m1_d = din("m1", [128, 2 * HID], BF16)
    wa_d = din("wa", [NACT + 1, F2], BF16)
    wq_d = din("wq", [128, 4], BF16)
    cols_d = din("cols", [128, 20], F32)
    onesm_d = din("onesmat_in", [128, 128], BF16)
    ident_d = din("ident_in", [128, 128], BF16)
    OUT = nc.dram_tensor("q", [1, PA], F32, kind="ExternalOutput").ap()

    with tile.TileContext(nc) as tc:
        with tc.tile_pool(name="w", bufs=1) as wp, \
             tc.tile_pool(name="eb", bufs=1) as ep, \
             tc.tile_pool(name="prp", bufs=3) as prp, \
             tc.tile_pool(name="csp", bufs=12) as csp, \
             tc.tile_pool(name="zp", bufs=4) as zp, \
             tc.tile_pool(name="s1p", bufs=9) as s1p, \
             tc.tile_pool(name="dp", bufs=3) as dp, \
             tc.tile_pool(name="uup", bufs=5) as uup, \
             tc.tile_pool(name="u2p", bufs=3) as u2p, \
             tc.tile_pool(name="vec", bufs=6) as vec, \
             tc.tile_pool(name="bcp", bufs=3) as bcp, \
             tc.tile_pool(name="tlp", bufs=4) as tlp, \
             tc.tile_pool(name="wvp", bufs=4) as wvp, \
             tc.tile_pool(name="sap", bufs=8) as sap, \
             tc.tile_pool(name="qvp", bufs=2) as qvp, \
             tc.tile_pool(name="ps", bufs=1, space="PSUM") as pp:

            T = nc.tensor; S = nc.scalar; D = nc.vector; G = nc.gpsimd
            SY = nc.sync

            # ---------- DMA: E blocks resident, spread across queues ----------
            # early phase is DMA-bandwidth-bound: each E block is half-split
            # across the sync + scalar queues in strict consumption order, so
            # block b lands as early as the aggregate bandwidth allows.
            # weights ride the gpsimd queue in first-use order.
            eb = [ep.tile([128, Kb[b] * BLK], BF16, name=f"eb{b}")
                  for b in range(NB)]
            def eslice(b, lo, hi):
                return (eb[b][:, lo * BLK:hi * BLK],
                        E_d[:, (boff[b] + lo) * BLK:(boff[b] + hi) * BLK])
            def ehalves(b):
                K = Kb[b]
                h1 = min(max(4, (K // 2 + 3) // 4 * 4), K)
                return h1, K
            def ehalves(b):
                K = Kb[b]
                h1 = min(max(4, (K // 2 + 3) // 4 * 4), K)
                return h1, K
            # all tiles first (names), then the interleaved issue schedule
            wgcn = wp.tile([128, HID], BF16)
            cols = wp.tile([128, 20], F32)
            actT = wp.tile([NACT + 1, PA], BF16)
            wa = wp.tile([NACT + 1, F2], BF16)
            wq = wp.tile([128, 4], BF16)
            w1 = wp.tile([128, 2 * F1], BF16)
            m1 = wp.tile([128, 2 * HID], BF16)
            onesm = wp.tile([128, 128], BF16)
            idm = wp.tile([128, 128], BF16)
            w2 = wp.tile([128, 8 * F2], BF16)
            def w2c(c2):
                return (w2[:, c2 * 1024:(c2 + 1) * 1024],
                        w2_d[:, c2 * 1024:(c2 + 1) * 1024])
            # sync+scalar queues: pure E in consumption order (half-split);
            # all weights ride the gpsimd queue.
            SY.dma_start(wgcn[:], wgcn_d[:])
            S.dma_start(cols[:], cols_d[:])
            for b in range(NB):
                h1 = ehalves(b)[0]
                SY.dma_start(*eslice(b, 0, h1))
                if h1 < Kb[b] and b < 6:
                    S.dma_start(*eslice(b, h1, Kb[b]))
            G.dma_start(w1[:], w1_d[:])
            G.dma_start(m1[:], m1_d[:])
            G.dma_start(onesm[:], onesm_d[:])
            G.dma_start(idm[:], ident_d[:])
            G.dma_start(actT[:], actT_d[:])
            G.dma_start(wa[:], wa_d[:])
            G.dma_start(wq[:], wq_d[:])
            for c2 in range(4):
                G.dma_start(*w2c(c2))
            for b in (6, 7):
                h1 = ehalves(b)[0]
                if h1 < Kb[b]:
                    G.dma_start(*eslice(b, h1, Kb[b]))
            bgcn = cols[:, 0:2]
            b1c = cols[:, 2:10]
            cvec = cols[:, 10:12]
            b2c = cols[:, 12:16]
            g2c = cols[:, 16:20]
            agg = wp.tile([128, PA], BF16)       # agg^T, feature-major

            # ---------- aggregation: bf16 pairwise trees, D majority ----------
            def agg_ops(b, pattern):
                """Closures (dependency-ordered) summing E block b -> agg."""
                K = Kb[b]; et = eb[b]
                av = agg[:, b * BLK:(b + 1) * BLK]
                ops = []
                sums = {"D": [], "G": []}
                nch = K // 4
                for q in range(nch):
                    eng = pattern[q % len(pattern)]
                    nm = "D" if eng is D else "G"
                    c3 = et[:, 4 * q * BLK:(4 * q + 4) * BLK].rearrange(
                        "p (s e) -> p s e", e=BLK)
                    pr = prp.tile([128, 2 * BLK], BF16, tag="pr" + nm)
                    p3 = pr[:].rearrange("p (s e) -> p s e", e=BLK)
                    ops.append(lambda eng=eng, p3=p3, c3=c3: eng.tensor_tensor(
                        p3, c3[:, 0::2, :], c3[:, 1::2, :], OP.add))
                    cs = csp.tile([128, BLK], BF16, tag="cs" + nm)
                    ops.append(lambda eng=eng, cs=cs, pr=pr: eng.tensor_tensor(
                        cs[:], pr[:, :BLK], pr[:, BLK:], OP.add))
                    sums[nm].append(cs)
                if K - 4 * nch == 2:
                    eng = pattern[nch % len(pattern)]
                    nm = "D" if eng is D else "G"
                    c3 = et[:, 4 * nch * BLK:(4 * nch + 2) * BLK].rearrange(
                        "p (s e) -> p s e", e=BLK)
                    cs = csp.tile([128, BLK], BF16, tag="cs" + nm)
                    ops.append(lambda eng=eng, cs=cs, c3=c3: eng.tensor_tensor(
                        cs[:], c3[:, 0, :], c3[:, 1, :], OP.add))
                    sums[nm].append(cs)
                parts = []
                for nm, eng in (("D", D), ("G", G)):
                    lst = sums[nm]
                    while len(lst) > 1:
                        nxt = []
                        for i in range(0, len(lst) - 1, 2):
                            t = csp.tile([128, BLK], BF16, tag="cs" + nm)
                            ops.append(lambda eng=eng, t=t, a=lst[i],
                                       b2=lst[i + 1]: eng.tensor_tensor(
                                           t[:], a[:], b2[:], OP.add))
                            nxt.append(t)
                        if len(lst) % 2:
                            nxt.append(lst[-1])
                        lst = nxt
                    if lst:
                        parts.append(lst[0])
                if len(parts) == 2:
                    ops.append(lambda a=parts[0], b2=parts[1]: D.tensor_tensor(
                        av, a[:], b2[:], OP.add))
                else:
                    ops.append(lambda a=parts[0]: D.tensor_scalar(
                        av, a[:], 0.0, None, OP.add))
                return ops

            def pe_agg_ops(b):
                """Aggregate block b on the tensor engine: identity-stationary
                accumulating matmuls over the K slots, one PSUM bank."""
                K = Kb[b]; et = eb[b]
                ps = {}
                ops = []
                for k in range(K):
                    def mmk(k=k, K=K, et=et):
                        if k == 0:
                            ps[0] = pp.tile([128, 512], F32, tag="qp",
                                            bufs=1, name="aggps")
                        T.matmul(ps[0][:, :BLK], idm[:],
                                 et[:, k * BLK:(k + 1) * BLK],
                                 start=(k == 0), stop=(k == K - 1))
                    ops.append(mmk)
                def drain(b=b):
                    S.activation(agg[:, b * BLK:(b + 1) * BLK],
                                 ps[0][:, :BLK], AF.Copy)
                ops.append(drain)
                return ops

            extras_by_pass = {
                0: agg_ops(2, (D, D, D, G)),
                1: agg_ops(3, (D, D, D, G)),
                2: agg_ops(4, (D, D, D, G)) + agg_ops(5, (D, D, D, G)),
                3: agg_ops(6, (D, D, D, G)) + agg_ops(7, (D, D, D, G)),
                4: [],
                5: [],
            }

            # ---------- head pass emitter (software pipelined) ----------
            def emit_z(pi):
                """z transform + relu for pass pi (hoisted into prev pass)."""
                s0, W, _ = PASSES[pi]
                zt = []
                for h in range(2):
                    zps = pp.tile([128, 512], F32, tag="mm", bufs=5,
                                  name="mmps")
                    T.matmul(zps[:, :W], wgcn[:, h * 128:(h + 1) * 128],
                             agg[:, s0:s0 + W], start=True, stop=True)
                    z = zp.tile([128, 512], BF16, tag="z")
                    S.activation(z[:, :W], zps[:, :W], AF.Relu,
                                 bias=bgcn[:, h:h + 1])
                    zt.append(z)
                return zt

            def emit_pass(pi, deferred, zt):
                s0, W, _ = PASSES[pi]
                extras = list(extras_by_pass[pi])
                exi = [0]
                npoints = [14]
                # early passes run while their extras' E blocks are still in
                # flight: weave extras only into the back of the pass so the
                # in-order vector stream never blocks on a pending DMA.
                active_from = {0: 7, 1: 10, 2: 9, 3: 9}.get(pi, 14)
                def pull():
                    npoints[0] -= 1
                    if npoints[0] >= active_from:
                        return
                    rem = len(extras) - exi[0]
                    if rem <= 0:
                        return
                    n = -(-rem // max(npoints[0] + 1, 1))
                    for _ in range(n):
                        if exi[0] < len(extras):
                            extras[exi[0]]()
                            exi[0] += 1
                defA = deferred.get("A", []); defB = deferred.get("B", [])
                defC = deferred.get("C", [])

                def mmt():
                    return pp.tile([128, 512], F32, tag="mm", bufs=5,
                                   name="mmps")

                for f in defA:
                    f()
                pull()

                # --- L1 ---
                s1r = []
                for c in range(8):
                    lp = mmt()
                    T.matmul(lp[:, :W], w1[:, c * 128:c * 128 + 128],
                             zt[0][:, :W], start=True, stop=False)
                    T.matmul(lp[:, :W], w1[:, F1 + c * 128:F1 + c * 128 + 128],
                             zt[1][:, :W], start=False, stop=True)
                    sr = s1p.tile([128, 512], BF16, tag="s1")
                    if c in S1_ON_D:
                        D.tensor_scalar(sr[:, :W], lp[:, :W], b1c[:, c:c + 1],
                                        0.0, OP.add, OP.max)
                    else:
                        S.activation(sr[:, :W], lp[:, :W], AF.Relu,
                                     bias=b1c[:, c:c + 1])
                    s1r.append(sr)
                    pull()
                    if c == 3:
                        for f in defB:
                            f()
                    if c == 5:
                        for f in defC:
                            f()

                # --- M1 quadratic-form stats ---
                ds = []
                for h in range(2):
                    mzp = mmt()
                    for kk in range(2):
                        T.matmul(mzp[:, :W],
                                 m1[:, kk * HID + h * 128:kk * HID + h * 128 + 128],
                                 zt[kk][:, :W], start=(kk == 0), stop=(kk == 1))
                    dd = dp.tile([128, 512], BF16, tag="ds")
                    D.scalar_tensor_tensor(dd[:, :W], mzp[:, :W],
                                           cvec[:, h:h + 1], zt[h][:, :W],
                                           OP.add, OP.mult)
                    ds.append(dd)
                pull()

                # --- L2 + LN1/LN2 stats (v-space: u = v/std1, rstd1 absorbed) ---
                vts = []
                wt = std1b = None
                for c2 in range(4):
                    lp2 = mmt()
                    for k8 in range(8):
                        T.matmul(lp2[:, :W],
                                 w2[:, c2 * 1024 + k8 * 128:c2 * 1024 + k8 * 128 + 128],
                                 s1r[k8][:, :W], start=(k8 == 0), stop=(k8 == 7))
                    if c2 == 0:
                        ps1 = pp.tile([128, 512], F32, tag="stat", bufs=1)
                        for h in range(2):
                            T.matmul(ps1[:, :W], onesm[:], ds[h][:, :W],
                                     start=(h == 0), stop=(h == 1))
                        wt = vec.tile([128, 512], F32, tag="vec")
                        D.tensor_scalar(wt[:, :W], ps1[:, :W], EPS / F1,
                                        EPS * (EPS + c1const / F1),
                                        OP.mult, OP.add)
                        std1b = vec.tile([128, 512], F32, tag="vec")
                        S.activation(std1b[:, :W], wt[:, :W], AF.Sqrt,
                                     scale=1.0 / EPS)
                    v = uup.tile([128, 512], F32, tag="v")
                    D.scalar_tensor_tensor(v[:, :W], std1b[:, :W],
                                           b2c[:, c2:c2 + 1], lp2[:, :W],
                                           OP.mult, OP.add)
                    v2 = u2p.tile([128, 512], BF16, tag="v2")
                    S.activation(v2[:, :W], v[:, :W], AF.Square)
                    vts.append((v, v2))
                    pull()

                # --- next pass's z hoisted here: its L1 starts immediately ---
                if pi + 1 < len(PASSES):
                    zt_next = emit_z(pi + 1)
                else:
                    zt_next = None

                # --- LN2 stats + tail ---
                pas = {}
                for c2 in range(1):
                    pa = pp.tile([128, 512], F32, tag="pa", bufs=1,
                                 name="paps")
                    T.matmul(pa[:, :W], wa[:, c2 * 128:(c2 + 1) * 128],
                             actT[:, s0:s0 + W], start=True, stop=True)
                    pas[c2] = pa
                ps2 = pp.tile([128, 512], F32, tag="stat", bufs=1)
                for c2 in range(4):
                    T.matmul(ps2[:, :W], onesm[:], vts[c2][1][:, :W],
                             start=(c2 == 0), stop=(c2 == 3))
                var2t = vec.tile([128, 512], F32, tag="vec")
                D.scalar_tensor_tensor(var2t[:, :W], ps2[:, :W], 1.0 / F2,
                                       wt[:, :W], OP.mult, OP.add)
                std2 = vec.tile([128, 512], F32, tag="vec")
                S.activation(std2[:, :W], var2t[:, :W], AF.Sqrt)
                rstd2b = bcp.tile([128, 512], F32, tag="rstd")
                D.reciprocal_approx_fast(rstd2b[:, :W], std2[:, :W])
                wvs = {}
                for c2 in range(4):
                    wv = wvp.tile([128, 512], F32, tag="wv")
                    G.tensor_tensor(wv[:, :W], vts[c2][0][:, :W],
                                    rstd2b[:, :W], OP.mult)
                    wvs[c2] = wv
                sas = {}
                for c2 in range(1):
                    t2 = tlp.tile([128, 512], F32, tag="t2")
                    D.scalar_tensor_tensor(t2[:, :W], wvs[c2][:, :W],
                                           g2c[:, c2:c2 + 1], pas[c2][:, :W],
                                           OP.mult, OP.add)
                    sa = sap.tile([128, 512], BF16, tag="sa")
                    S.activation(sa[:, :W], t2[:, :W], AF.Relu)
                    sas[c2] = sa

                def tail_A():
                    for c2 in (1, 2, 3):
                        pa = pp.tile([128, 512], F32, tag="pa", bufs=1,
                                     name="paps")
                        T.matmul(pa[:, :W], wa[:, c2 * 128:(c2 + 1) * 128],
                                 actT[:, s0:s0 + W], start=True, stop=True)
                        t2 = tlp.tile([128, 512], F32, tag="t2")
                        D.scalar_tensor_tensor(t2[:, :W], wvs[c2][:, :W],
                                               g2c[:, c2:c2 + 1],
                                               pa[:, :W], OP.mult, OP.add)
                        sa = sap.tile([128, 512], BF16, tag="sa")
                        S.activation(sa[:, :W], t2[:, :W], AF.Relu)
                        sas[c2] = sa
                qps = {}
                def tail_B():
                    qp = pp.tile([128, 512], F32, tag="qp", bufs=1, name="qp")
                    for c2 in range(4):
                        T.matmul(qp[0:1, :W], wq[:, c2:c2 + 1], sas[c2][:, :W],
                                 start=(c2 == 0), stop=(c2 == 3))
                    qps[0] = qp
                def tail_C():
                    qv = qvp.tile([1, 512], F32, tag="qv")
                    S.activation(qv[:, :W], qps[0][0:1, :W], AF.Copy, bias=bq)
                    SY.dma_start(OUT[:, s0:s0 + W], qv[:, :W])
                return {"A": [tail_A], "B": [tail_B], "C": [tail_C]}, zt_next

            # ---------- emission ----------
            for f in agg_ops(0, (D,)):
                f()
            for f in agg_ops(1, (D,)):
                f()
            deferred = {}
            zt = emit_z(0)
            for pi in range(len(PASSES)):
                deferred, zt = emit_pass(pi, deferred, zt)
            for f in deferred["A"] + deferred["B"] + deferred["C"]:
                f()
    nc.compile()
    return nc


def kernel(**inputs):
    weights, percore, rows_list, meta = _preprocess(**inputs)

    key = (meta["Kb"], meta["tot_cols"])
    if key not in _KERNEL_CACHE:
        _KERNEL_CACHE[key] = _build(meta)
    nc = _KERNEL_CACHE[key]

    in_maps = []
    for c in range(N_CORES):
        m = dict(weights)
        m["E"] = percore["E"][c]
        m["actT"] = percore["actT"][c]
        in_maps.append(m)

    trace = os.environ.get("KERNEL_TRACE", "0") == "1"
    kw = {}
    if trace:
        import types, contextlib, ctypes
        if "antenv.axon_hooks" not in sys.modules:
            lib = ctypes.CDLL("/opt/axon/libaxon_pjrt.so")
            lib.axon_start_nrt_profile.argtypes = [
                ctypes.POINTER(ctypes.c_int64), ctypes.c_size_t]
            lib.axon_start_nrt_profile.restype = ctypes.c_int64
            lib.axon_stop_nrt_profile.argtypes = [ctypes.c_char_p]
            lib.axon_stop_nrt_profile.restype = ctypes.c_int64

            @contextlib.contextmanager
            def _hook(output_dir, device_ids):
                import jax
                jax.devices()
                if device_ids:
                    ids = (ctypes.c_int64 * len(device_ids))(*device_ids)
                    rc = lib.axon_start_nrt_profile(ids, len(device_ids))
                else:
                    rc = lib.axon_start_nrt_profile(None, 0)
                if rc != 0:
                    raise RuntimeError(f"axon_start_nrt_profile rc={rc}")
                try:
                    yield
                finally:
                    n = lib.axon_stop_nrt_profile(str(output_dir).encode())
                    print(f"profile: {n} file(s) written to {output_dir}",
                          file=sys.stderr)

            mod = types.ModuleType("antenv.axon_hooks")
            mod.get_axon_ntff_profile_hook = lambda: _hook
            sys.modules["antenv.axon_hooks"] = mod
        kw = dict(trace=True,
                  tmpdir=os.environ.get("KERNEL_TRACE_DIR") or None)

    res = run_bass_kernel_spmd(nc, in_maps, list(range(N_CORES)), **kw)
    if trace and res.exec_time_ns is not None:
        print(f"HW exec time: {res.exec_time_ns} ns")

    out = np.empty((N_AGENTS, 1), np.float32)
    for c in range(N_CORES):
        q = res.results[c]["q"].reshape(PA)
        out[rows_list[c], 0] = q
    return out
